# revision 50
# baseline (speedup 1.0000x reference)
"""MHSA block (b=8, c=256, h=w=32, nh=8) on 8 Trainium2 cores.

Sharding: pure data parallel -- one batch element per NeuronCore, no
collectives.  Per-core algorithm (X = x[b] as (C=256, L=1024)):

  QK   = Wqk @ X  (q-part pre-scaled by A = scale*log2e*128; bias via
                   Identity+biasAP psum->sbuf copies)                (512, L)
  ktp/qtp = per-duo zero-padded K/Q tiles (head A rows 0:32, head B rows
            64:96) built by DVE copies
  V^T  = X^T @ WvT_pad  (64-col head blocks: 32 V + ones col + zeros) (L, 512)

Main pipeline: 64 iterations, one per duo (tg, head-pair mp, ih) x jc:
  S^T  = two CONCURRENT K=64 row-group matmuls (rows 0:64 / 64:128 of the
         padded tiles; zero rows null the partner head) -> one (128,1024)
         PSUM tile, double-buffered.  K>=64 pairs clock all 128 PE rows,
         which holds the HAM clock-gate at 2.4 GHz -- K=32 matmuls starve
         its utilization monitor and throttle the whole kernel to 1.2 GHz.
  P^T  = exp, alternating engines per iteration (any same-engine run >1
         stalls the st rotation):
           ACT: real exp (scale=ln2/128)
           DVE: Schraudolph fast-exp rint(t + B_SCHRAUD) -> int16 whose
                bits ARE bf16 (rel err ~3%, washes out after the softmax
                normalization because the denominators are summed from
                the same approximated P)
  PV   = 2 col-group matmuls (LAG=3 behind) accumulate [O_h; l_h] into a
         pair-packed (128,512) PSUM bank (rows 0:33 / 64:97; jc==0 uses
         the 64-wide zero-padded lhsT to sanitize pad rows)

A 12-matmul dummy warmup burst during the input-DMA wait pre-warms the
HAM; vt chunks 2-7 are hooked into early iterations so their matmuls fill
exp-wait gaps instead of delaying the first S^T.

Per-duo norm chain in 2 stages spread over the next duo (all inputs ready
when emitted, so no engine queue-head stalls):
  s0 (at pv jc7): pv -> o_sb copy (frees the bank; rows 32/96 are the
      softmax denominators via the V ones-columns) + denominator-row DMAs
  s1: 1/l via reciprocal_approx_fast emitted straight to bf16 (custom-DVE
      call; output converter does the downcast)
  s2: rp = K=2 matmul broadcast of 1/l (borrows a pv psum slot);
      on = o_sb * rp (DVE); wp matmuls accumulate into a SHARED pj psum
      tile per ih half; x enters via an identity matmul (first chain);
      resid bias (bproj + Wproj@bv) via the final Identity+biasAP copies
      (last chain) -> out DMA

PSUM budget (8 banks): st 2x(128,1024)=4 (qk chunks and warmup borrow),
pv 2x(128,512)=2 (vt chunks and rp borrow), pj 1x(128,1024)=2.
"""

import sys
import os

sys.path.insert(0, "/opt/trn_rl_repo")

from contextlib import ExitStack

import numpy as np

NH, DH, C, L = 8, 32, 256, 1024
B = 8
SCALE = DH ** -0.5
N_CORES = 8

LOG2E = 1.4426950408889634
A_FOLD = SCALE * LOG2E * 128.0          # folded into Wq / bq host-side
ACT_SCALE = float(np.log(2.0) / 128.0)  # ACT exp reads pre-scaled scores
B_SCHRAUD = 127.0 * 128.0 - 0.043 * 128.0  # rint() semantics on HW

# ---- engine-assignment knobs ----
# exp engine per iteration (64): alternate ACT/DVE, a few extra to ACT
# because DVE's PSUM read is ~25% slower than ACT's.
EXP_ENG = ["a" if it % 2 == 0 else "d" for it in range(64)]
for _it in (21, 53):
    EXP_ENG[_it] = "a"
O_ENG = ["act"] * 8               # per-duo pv->o_sb copy engine
VT_ENG = ["act"] * 8              # startup vt copies
QK_ENG = ["act", "act", "act", "act"]
FIN_ENG = [["act", "act"], ["act", "dve"]]  # per-(ih, mt2) final copies
RL_ENG = "fused"                    # 1/l f32->bf16 cast (chain critical path)

_CACHE = {}


def _build_nc():
    import concourse.tile as tile
    from concourse import bacc, mybir

    f32 = mybir.dt.float32
    bf16 = mybir.dt.bfloat16
    i16 = mybir.dt.int16
    Exp = mybir.ActivationFunctionType.Exp
    Identity = mybir.ActivationFunctionType.Identity

    nc = bacc.Bacc("TRN2", target_bir_lowering=False, debug=False)

    XW_COLS = 5248
    xw_d = nc.dram_tensor("xw", [128, XW_COLS], bf16, kind="ExternalInput").ap()
    bc_d = nc.dram_tensor("bc", [128, 6], f32, kind="ExternalInput").ap()
    e_d = nc.dram_tensor("ee", [2, 128], bf16, kind="ExternalInput").ap()
    out_d = nc.dram_tensor("out", [C, L], f32, kind="ExternalOutput").ap()

    with tile.TileContext(nc) as tc, ExitStack() as ctx:
        persist = ctx.enter_context(tc.tile_pool(name="persist", bufs=1))
        ptpa = ctx.enter_context(tc.tile_pool(name="pta", bufs=4))
        ptpb = ctx.enter_context(tc.tile_pool(name="ptb", bufs=4))
        onpool = ctx.enter_context(tc.tile_pool(name="on", bufs=2))
        smallp = ctx.enter_context(tc.tile_pool(name="small", bufs=2))
        finp = ctx.enter_context(tc.tile_pool(name="fin", bufs=2))
        stps = ctx.enter_context(tc.tile_pool(name="stps", bufs=2, space="PSUM"))
        pvps = ctx.enter_context(tc.tile_pool(name="pvps", bufs=2, space="PSUM"))
        pjps = ctx.enter_context(tc.tile_pool(name="pjps", bufs=1, space="PSUM"))

        xw = persist.tile([128, XW_COLS], bf16, tag="xw", name="xw")
        nc.sync.dma_start(xw[:, 0:3072], xw_d[:, 0:3072])
        x_sb = [xw[:, 0:1024], xw[:, 1024:2048]]
        wqk_sb = [xw[:, 2048:2560], xw[:, 2560:3072]]
        wv_sb = [xw[:, 3072:3584], xw[:, 3584:4096]]
        wp_sb = [xw[:, 4096 + 256 * p:4096 + 256 * (p + 1)] for p in range(4)]
        id_sb = xw[:, 5120:5248]

        bc_sb = persist.tile([128, 6], f32, tag="bc", name="bc")
        nc.sync.dma_start(bc_sb[:], bc_d[:])

        warm = persist.tile([1, 8], f32, tag="warm", name="warm")
        nc.gpsimd.memset(warm[:], 0.0)
        nc.scalar.activation(warm[:], warm[:], Exp)

        e_sb = persist.tile([2, 128], bf16, tag="ee", name="ee")
        nc.sync.dma_start(e_sb[:], e_d[:])

        # PE warmup burst: dense K=128 dummy matmuls during the input-DMA
        # wait pull the HAM clock-gate to 2.4 GHz before QK starts.
        dummy = persist.tile([128, 512], bf16, tag="dummy", name="dummy")
        nc.vector.memset(dummy[:], 0.0)
        wps = stps.tile([128, L], f32, tag="st", name="warmps")
        for i in range(12):
            nc.tensor.matmul(wps[:, 0:512], lhsT=dummy[:, 0:128],
                             rhs=dummy[:], start=True, stop=True)

        def psum_to_sbuf(engine, dst, src, bias=None):
            if engine == "act":
                if bias is None:
                    nc.scalar.copy(dst, src)
                else:
                    nc.scalar.activation(dst, src, Identity, bias=bias)
            else:
                if bias is None:
                    nc.vector.tensor_copy(dst, src)
                else:
                    nc.vector.tensor_scalar_add(dst, src, bias)

        # ---- QK gemm:  QK(512, L) = WqkT.T @ X; bias added on the copy ----
        qk_sb = [None] * 4

        def qk_chunk(mt):
            ps = stps.tile([128, L], f32, tag="st", name="qkps")
            for nh_ in range(2):
                o = ps[:, nh_ * 512:(nh_ + 1) * 512]
                for kt in range(2):
                    nc.tensor.matmul(
                        o,
                        lhsT=wqk_sb[kt][:, mt * 128:(mt + 1) * 128],
                        rhs=x_sb[kt][:, nh_ * 512:(nh_ + 1) * 512],
                        start=(kt == 0),
                        stop=(kt == 1),
                    )
            qk = persist.tile([128, L], bf16, tag=f"qk{mt}", name=f"qk{mt}")
            psum_to_sbuf(QK_ENG[mt], qk[:], ps[:], bias=bc_sb[:, mt:mt + 1])
            qk_sb[mt] = qk

        # ---- V^T gemm: VT(L, 512) = X.T @ WvT_pad  (64-col head blocks) ----
        vt_sb = [None] * 8

        def vt_chunk(jt):
            ps = pvps.tile([128, 512], f32, tag="pv", name="vtps")
            for kt in range(2):
                nc.tensor.matmul(
                    ps[:],
                    lhsT=x_sb[kt][:, jt * 128:(jt + 1) * 128],
                    rhs=wv_sb[kt],
                    start=(kt == 0),
                    stop=(kt == 1),
                )
            vt = persist.tile([128, 512], bf16, tag=f"vt{jt}", name=f"vt{jt}")
            psum_to_sbuf(VT_ENG[jt], vt[:], ps[:])
            ones_cols = vt[:].rearrange("p (h c) -> p h c", h=8)[:, :, 32:33]
            nc.gpsimd.memset(ones_cols, 1.0)
            vt_sb[jt] = vt

        # per-duo zero-padded K/Q tiles: head A at rows 0:32, head B at rows
        # 64:96, zeros elsewhere.  The two S^T matmuls run as a CONCURRENT
        # K=64 row-group pair (rows 0:64 / 64:128) writing the two banks of
        # one st tile; together they clock all 128 PE rows, which keeps the
        # HAM clock-gate at 2.4 GHz (low-K matmuls throttle it).
        # A-heads (2mp) sit at qk rows 64mp -- aligned with tile_position
        # (64mp,0), so their rhs reads qk_sb directly (garbage partner rows
        # are nulled by the ktpA zeros).  Only B-heads (2mp+1) need moved
        # copies, packed cross-wise so MM-B lands on row group 64(1-mp).
        ktpA = [persist.tile([128, L], bf16, tag=f"ktpA{tg}", name=f"ktpA{tg}")
                for tg in range(2)]
        ktpB = [persist.tile([128, L], bf16, tag=f"ktpB{tg}", name=f"ktpB{tg}")
                for tg in range(2)]
        qtpB = [persist.tile([128, L], bf16, tag=f"qtpB{tg}", name=f"qtpB{tg}")
                for tg in range(2)]
        for tg in range(2):
            nc.gpsimd.memset(ktpA[tg][:], 0.0)
            nc.gpsimd.memset(ktpB[tg][:], 0.0)
            nc.gpsimd.memset(qtpB[tg][:], 0.0)

        def kt_pad(tg, mps=(0, 1)):
            for mp in mps:
                ra = 64 * mp           # A-head row block
                rb = 64 * (1 - mp)     # B-head target row block
                nc.vector.tensor_copy(
                    ktpA[tg][ra:ra + 32, :],
                    qk_sb[2 + tg][64 * mp:64 * mp + 32, :])
                nc.vector.tensor_copy(
                    ktpB[tg][rb:rb + 32, :],
                    qk_sb[2 + tg][64 * mp + 32:64 * mp + 64, :])
                nc.vector.tensor_copy(
                    qtpB[tg][rb:rb + 32, :],
                    qk_sb[tg][64 * mp + 32:64 * mp + 64, :])

        qk_chunk(0)
        qk_chunk(2)
        kt_pad(0, mps=(0,))
        nc.sync.dma_start(xw[:, 3072:4160], xw_d[:, 3072:4160])
        nc.sync.dma_start(xw[:, 4160:5248], xw_d[:, 4160:5248])
        # vt2-7 hook into the first iterations so their MMs fill the PE
        # during exp waits instead of serially delaying the first S^T
        vt_chunk(0)
        vt_chunk(1)

        # ---- main pipeline ----
        # duo = (tg, mp, ih): heads (4tg+2mp, 4tg+2mp+1), i-cols ih half
        duos = []
        for ih in range(2):
            for tg in range(2):
                for mp in range(2):
                    duos.append((tg, mp, ih))

        pj_ref = [None, None]   # shared proj psum per ih
        defer1 = []             # chain stage 1 (1/l serial path)
        defer2 = []             # chain stage 2 (bcast/proj, needs stage 1)

        def make_duo(d, tg, mp, ih):
            qt = qk_sb[tg]
            cols = slice(ih * 512, (ih + 1) * 512)
            state = {}

            def st_fn(jc):
                it = 8 * d + jc
                for h in iter_hooks.pop(it, []):
                    h()
                if jc in (2, 6) and defer1:
                    defer2.append(defer1.pop(0)())
                if jc in (4, 0) and defer2:
                    defer2.pop(0)()
                st = stps.tile([128, L], f32, tag="st", name="st")
                ra = 64 * mp
                rb = 64 * (1 - mp)
                nc.tensor.matmul(
                    st[:, 0:512],
                    lhsT=ktpA[tg][ra:ra + 64, jc * 128:(jc + 1) * 128],
                    rhs=qt[ra:ra + 64, cols],
                    start=True,
                    stop=True,
                    tile_position=(ra, 0),
                )
                nc.tensor.matmul(
                    st[:, 512:1024],
                    lhsT=ktpB[tg][rb:rb + 64, jc * 128:(jc + 1) * 128],
                    rhs=qtpB[tg][rb:rb + 64, cols],
                    start=True,
                    stop=True,
                    tile_position=(rb, 0),
                )
                if EXP_ENG[it] == "a":
                    pt = ptpa.tile([128, L], bf16, tag="pt", name="ptA")
                    nc.scalar.activation(pt[:], st[:], Exp, scale=ACT_SCALE)
                    state[jc] = pt[:]
                else:
                    pt = ptpb.tile([128, L], i16, tag="pt", name="ptB")
                    nc.vector.tensor_scalar_add(pt[:], st[:], B_SCHRAUD)
                    state[jc] = pt[:].bitcast(bf16)

            def pv_fn(jc):
                if jc == 0:
                    state["pv"] = pvps.tile([128, 512], f32, tag="pv", name="pv")
                rhs2 = state.pop(jc)
                pv = state["pv"]
                for s in range(2):
                    h = 4 * tg + 2 * mp + s
                    rhs = rhs2[:, s * 512:s * 512 + 512]
                    if jc == 0:
                        nc.tensor.matmul(
                            pv[64 * s:64 * s + 64, :],
                            lhsT=vt_sb[0][:, h * 64:h * 64 + 64],
                            rhs=rhs,
                            start=True,
                            stop=False,
                            tile_position=(0, 64 * s),
                        )
                    else:
                        nc.tensor.matmul(
                            pv[64 * s:64 * s + 33, :],
                            lhsT=vt_sb[jc][:, h * 64:h * 64 + 33],
                            rhs=rhs,
                            start=False,
                            stop=(jc == 7),
                            tile_position=(0, 64 * s),
                        )
                if jc == 7:
                    o_t = persist.tile([128, 512], f32, tag=f"o{d}", name=f"o{d}")
                    psum_to_sbuf(O_ENG[d], o_t[:], pv[:])
                    l_sb = smallp.tile([2, 512], f32, tag="l", name="l")
                    nc.sync.dma_start(l_sb[0:1, :], o_t[32:33, :])
                    nc.sync.dma_start(l_sb[1:2, :], o_t[96:97, :])
                    defer1.append(make_chain(d, tg, mp, ih, cols, o_t, l_sb))

            return st_fn, pv_fn

        def make_chain(d, tg, mp, ih, cols, o_t, l_sb):
            first = (tg == 0 and mp == 0)
            last = (tg == 1 and mp == 1)

            def stage1():
                rl = smallp.tile([2, 512], bf16, tag="rl", name="rl")
                if RL_ENG == "fused":
                    # reciprocal_approx_fast with bf16 out: bypass the f32-out
                    # assert (input stays f32; the output converter downcasts)
                    from concourse.dve_ops import (
                        RECIP_APPROX_FAST_CONSTS as _rc,
                        RECIPROCAL_APPROX_FAST as _rop,
                    )
                    nc.vector._custom_dve(_rop, out=rl[:], in0=l_sb[:],
                                          s0=_rc["s0"], s1=_rc["s1"],
                                          imm2=_rc["imm2"])
                else:
                    rl32 = smallp.tile([2, 512], f32, tag="rl32", name="rl32")
                    nc.vector.reciprocal_approx_fast(rl32[:], l_sb[:])
                    nc.vector.tensor_copy(rl[:], rl32[:])

                def stage2():
                    rp = pvps.tile([128, 512], f32, tag="pv", name="rp")
                    nc.tensor.matmul(rp[:], lhsT=e_sb[:], rhs=rl[:],
                                     start=True, stop=True)
                    on = onpool.tile([128, 512], bf16, tag="on", name="on")
                    nc.vector.tensor_mul(on[:], o_t[:], rp[:])

                    if first:
                        pj_ref[ih] = pjps.tile([128, L], f32, tag="pj",
                                               name="pj")
                    pj = pj_ref[ih]
                    for mt2 in range(2):
                        pjv = pj[:, mt2 * 512:(mt2 + 1) * 512]
                        if first:
                            nc.tensor.matmul(pjv, lhsT=id_sb,
                                             rhs=x_sb[mt2][:, cols],
                                             start=True, stop=False)
                        nc.tensor.matmul(
                            pjv,
                            lhsT=wp_sb[2 * tg + mp][:, mt2 * 128:(mt2 + 1) * 128],
                            rhs=on[:],
                            start=False,
                            stop=last,
                        )
                        if last:
                            # halved fins: each half's out-DMA overlaps the
                            # next half's copy, shortening the drain tail
                            fin = finp.tile([128, 512], f32, tag=f"fin{mt2}",
                                            name="fin")
                            c0 = ih * 512
                            for hh in range(2):
                                fh = slice(256 * hh, 256 * (hh + 1))
                                psum_to_sbuf(
                                    FIN_ENG[ih][mt2], fin[:, fh],
                                    pj[:, mt2 * 512 + 256 * hh:
                                       mt2 * 512 + 256 * (hh + 1)],
                                    bias=bc_sb[:, 4 + mt2:5 + mt2])
                                nc.sync.dma_start(
                                    out_d[mt2 * 128:(mt2 + 1) * 128,
                                          c0 + 256 * hh:c0 + 256 * (hh + 1)],
                                    fin[:, fh])

                return stage2

            return stage1

        iter_hooks = {
            1: [lambda: vt_chunk(2), lambda: vt_chunk(3)],
            2: [lambda: vt_chunk(4), lambda: vt_chunk(5),
                lambda: kt_pad(0, mps=(1,))],
            3: [lambda: vt_chunk(6), lambda: vt_chunk(7)],
            6: [lambda: qk_chunk(1)],
            9: [lambda: qk_chunk(3)],
            11: [lambda: kt_pad(1)],
        }
        LAG = 3
        pv_queue = []
        for d, (tg, mp, ih) in enumerate(duos):
            st_fn, pv_fn = make_duo(d, tg, mp, ih)
            for jc in range(8):
                st_fn(jc)
                pv_queue.append((pv_fn, jc))
                if len(pv_queue) > LAG:
                    pj_fn, pjc = pv_queue.pop(0)
                    pj_fn(pjc)
        for pj_fn, pjc in pv_queue:
            pj_fn(pjc)
            if defer1:
                defer2.append(defer1.pop(0)())
        while defer1:
            defer2.append(defer1.pop(0)())
        while defer2:
            defer2.pop(0)()

    nc.compile()
    return nc


def _get_nc():
    if "nc" not in _CACHE:
        _CACHE["nc"] = _build_nc()
    return _CACHE["nc"]


def _pack_weights(w_qkv, b_qkv, w_proj, b_proj):
    w_qkv = np.asarray(w_qkv, dtype=np.float32)
    b_qkv = np.asarray(b_qkv, dtype=np.float32)
    w_proj = np.asarray(w_proj, dtype=np.float32)
    b_proj = np.asarray(b_proj, dtype=np.float32)

    wqkT = np.ascontiguousarray(w_qkv[:512].T)                  # (256, 512)
    wqkT[:, 0:256] *= A_FOLD
    bqk = b_qkv[:512].copy()
    bqk[0:256] *= A_FOLD
    bqkc = np.ascontiguousarray(bqk.reshape(4, 128).T)          # (128, 4)

    wvT = np.zeros((C, 512), dtype=np.float32)
    for h in range(NH):
        wvT[:, h * 64:h * 64 + 32] = w_qkv[512 + h * 32:512 + (h + 1) * 32].T

    wpT = np.zeros((512, 256), dtype=np.float32)
    for p in range(4):
        wpT[p * 128 + 0:p * 128 + 32, :] = w_proj[:, (2 * p) * 32:(2 * p + 1) * 32].T
        wpT[p * 128 + 64:p * 128 + 96, :] = w_proj[:, (2 * p + 1) * 32:(2 * p + 2) * 32].T

    ee = np.zeros((2, 128), dtype=np.float32)
    ee[0, 0:32] = 1.0
    ee[1, 64:96] = 1.0

    resid_bias = b_proj + w_proj @ b_qkv[512:768]
    bc = np.zeros((128, 6), dtype=np.float32)
    bc[:, 0:4] = bqkc
    bc[:, 4] = resid_bias[0:128]
    bc[:, 5] = resid_bias[128:256]

    wblob = np.zeros((128, 3200), dtype=np.float32)
    wblob[:, 0:512] = wqkT[0:128]
    wblob[:, 512:1024] = wqkT[128:256]
    wblob[:, 1024:1536] = wvT[0:128]
    wblob[:, 1536:2048] = wvT[128:256]
    for p in range(4):
        wblob[:, 2048 + 256 * p:2048 + 256 * (p + 1)] = wpT[p * 128:(p + 1) * 128]
    wblob[:, 3072:3200] = np.eye(128, dtype=np.float32)
    return wblob, ee, bc


def _bf16(a):
    import ml_dtypes

    return np.asarray(a).astype(ml_dtypes.bfloat16)


def _install_ntff_hook_module():
    """bass_utils wants antenv.axon_hooks for trace=True under axon; this
    image's antenv lacks it.  Inject an equivalent module into sys.modules."""
    if "antenv.axon_hooks" in sys.modules:
        return
    try:
        import antenv.axon_hooks  # noqa: F401

        return
    except ImportError:
        pass
    import contextlib
    import ctypes
    import types

    mod = types.ModuleType("antenv.axon_hooks")
    state = {"hook": None, "inited": False}

    def _default_hook():
        so_path = "/opt/axon/libaxon_pjrt.so"
        if not os.path.exists(so_path):
            return None
        lib = ctypes.CDLL(so_path)
        if not hasattr(lib, "axon_start_nrt_profile"):
            return None
        lib.axon_start_nrt_profile.argtypes = [
            ctypes.POINTER(ctypes.c_int64),
            ctypes.c_size_t,
        ]
        lib.axon_start_nrt_profile.restype = ctypes.c_int64
        lib.axon_stop_nrt_profile.argtypes = [ctypes.c_char_p]
        lib.axon_stop_nrt_profile.restype = ctypes.c_int64

        @contextlib.contextmanager
        def _hook(output_dir, device_ids):
            import jax

            jax.devices()
            if device_ids:
                ids = (ctypes.c_int64 * len(device_ids))(*device_ids)
                rc = lib.axon_start_nrt_profile(ids, len(device_ids))
            else:
                rc = lib.axon_start_nrt_profile(None, 0)
            if rc != 0:
                raise RuntimeError(f"axon_start_nrt_profile rc={rc}")
            try:
                yield
            finally:
                n = lib.axon_stop_nrt_profile(str(output_dir).encode())
                if n < 0:
                    raise RuntimeError(f"axon_stop_nrt_profile rc={n}")
                print(f"profile: {n} file(s) written to {output_dir}")

        return _hook

    def set_axon_ntff_profile_hook(hook):
        state["hook"] = hook
        state["inited"] = True

    def get_axon_ntff_profile_hook():
        if not state["inited"]:
            state["hook"] = _default_hook()
            state["inited"] = True
        return state["hook"]

    mod.set_axon_ntff_profile_hook = set_axon_ntff_profile_hook
    mod.get_axon_ntff_profile_hook = get_axon_ntff_profile_hook
    sys.modules["antenv.axon_hooks"] = mod


def _prepare_in_maps(x, w_qkv, b_qkv, w_proj, b_proj):
    x = np.asarray(x, dtype=np.float32)
    b, c, h, w = x.shape
    assert (b, c, h, w) == (B, C, 32, 32)

    wblob, ee, bc = _pack_weights(w_qkv, b_qkv, w_proj, b_proj)
    wblob_bf = _bf16(wblob)
    ee_bf = _bf16(ee)
    bc = np.ascontiguousarray(bc, dtype=np.float32)

    in_maps = []
    for core in range(N_CORES):
        xm = np.ascontiguousarray(x[core].reshape(C, L))
        xw = np.empty((128, 5248), dtype=wblob_bf.dtype)
        xw[:, 0:1024] = _bf16(xm[0:128])
        xw[:, 1024:2048] = _bf16(xm[128:256])
        xw[:, 2048:5248] = wblob_bf
        in_maps.append(dict(xw=xw, bc=bc, ee=ee_bf))
    return in_maps


def kernel(x, w_qkv, b_qkv, w_proj, b_proj, _trace=False, _trace_kwargs=None):
    if _trace:
        _install_ntff_hook_module()
    from concourse.bass_utils import run_bass_kernel_spmd

    in_maps = _prepare_in_maps(x, w_qkv, b_qkv, w_proj, b_proj)
    nc = _get_nc()

    res = run_bass_kernel_spmd(
        nc,
        in_maps,
        list(range(N_CORES)),
        trace=_trace,
        **(_trace_kwargs or {}),
    )
    out = np.stack([res.results[core]["out"] for core in range(N_CORES)])
    if _trace:
        _CACHE["last_result"] = res
    return out.reshape(B, C, 32, 32)
